# revision 27
# baseline (speedup 1.0000x reference)
# Swin-style window attention (B=256 windows, N=196, C=768, H=12) on 8 trn2
# cores. Data-parallel over windows: 32 windows/core, chunks of 4 windows.
#
# Schedule: everything is woven at (window, head-pair) granularity so no
# engine ever drains. Per chunk: q0/k0 + V(w0) up front, then 24 attention
# pairs with background work attached to each pair slot:
#   w0 pairs carry the remaining 10 qk block GEMMs,
#   w1-w3 pairs carry prev-chunk proj slices (per 128-out-block x window,
#   24 of them), V(w1..w3), x-prefetch for the next chunk, finish-muls and
#   denominator chains.
# Engine assignment: q-copies/exp/proj/V-copies on Scalar, k-copies and
# finish muls and AV casts on DVE, den gather on GpSimd, weight DMAs spread
# over gpsimd/vector queues at startup so the Sync queue starts x/wqk
# immediately. ~28 warmup matmuls run during the initial DMA wait so the
# PE HAM clock-gate is at 2.4 GHz when real work arrives.
#
# Attention per (window, head): S.T = kT_h.T @ qT_h into a per-head
# [128,392] PSUM bank tile (keys 0:128 cols 0:196, keys 128:196 cols
# 196:392), p = exp(0.125*S) * exp(rpb) (one ACT + one DVE mul, 392 cols),
# OT = [V_h | ones].T @ p pair-packed into [65, 392] PSUM (den row 64),
# 1/den via gpsimd strided gather + reciprocal + DRAM-bounce broadcast.
# v_bias is folded into the proj bias on the host (softmax rows sum to 1).
# PSUM banks: mm [128,512]x3, st [128,392]x3, ot [65,392]x2 = 8.
import sys

sys.path.insert(0, "/opt/trn_rl_repo")

from contextlib import ExitStack

import ml_dtypes
import numpy as np

import concourse.bass as bass
import concourse.bacc as bacc
import concourse.mybir as mybir
import concourse.tile as tile
from concourse.bass_utils import run_bass_kernel_spmd

F32 = mybir.dt.float32
BF16 = mybir.dt.bfloat16
AF = mybir.ActivationFunctionType
ALU = mybir.AluOpType

_NC_CACHE = {}
NCORES = 8
B, N, C, H = 256, 196, 768, 12
HD = C // H  # 64
WPC = B // NCORES  # 32 windows per core
T = WPC * N  # 6272 tokens per core
CB = C // 128  # 6 contraction blocks
CHUNK_W = 4  # windows per chunk
VG = HD + 2  # 66: v col group stride (64 v cols + ones col + pad)
VW = H * VG  # 792: strided V tile width


def _install_ntff_hook():
    """Recreate the antenv.axon_hooks shim so trace=True works under axon."""
    import types

    if "antenv.axon_hooks" in sys.modules:
        return
    mod = types.ModuleType("antenv.axon_hooks")
    mod._hook = None
    mod.set_axon_ntff_profile_hook = lambda h: setattr(mod, "_hook", h)
    mod.get_axon_ntff_profile_hook = lambda: mod._hook
    sys.modules["antenv.axon_hooks"] = mod
    try:
        sys.path.insert(0, "/root/.axon_site/trn_agent_boot")
        from trn_boot import _ntff_profile_via_ctypes

        hook = _ntff_profile_via_ctypes("/opt/axon/libaxon_pjrt.so")
        if hook is not None:
            mod._hook = hook
    except Exception:
        pass


def _build_nc(wpc=WPC, chunk_w=CHUNK_W):
    t_total = wpc * N
    nchunk = wpc // chunk_w
    chunk_t = chunk_w * N

    nc = bacc.Bacc("TRN2", target_bir_lowering=False, debug=False,
                   num_devices=NCORES)
    xT_d = nc.dram_tensor("xT", [C, t_total], BF16, kind="ExternalInput").ap()
    wqk_d = nc.dram_tensor("wqkT", [C, 2 * C], BF16, kind="ExternalInput").ap()
    wv_d = nc.dram_tensor("wvT", [C, C], BF16, kind="ExternalInput").ap()
    wp_d = nc.dram_tensor("projwT", [C, C], BF16, kind="ExternalInput").ap()
    qb_d = nc.dram_tensor("qbT", [128, CB], F32, kind="ExternalInput").ap()
    pb_d = nc.dram_tensor("pbT", [128, CB], F32, kind="ExternalInput").ap()
    erp_d = nc.dram_tensor("erpT", [H // 2, 128, 4 * N], BF16,
                           kind="ExternalInput").ap()
    rb_scr = nc.dram_tensor("rb_scr", [2, H * N], BF16, kind="Internal").ap()
    yT_d = nc.dram_tensor("yT", [C, t_total], F32, kind="ExternalOutput").ap()

    with tile.TileContext(nc) as tc, ExitStack() as ctx:
        const = ctx.enter_context(tc.tile_pool(name="const", bufs=1))
        wpool = ctx.enter_context(tc.tile_pool(name="w", bufs=1))
        xpool = ctx.enter_context(tc.tile_pool(name="x", bufs=2))
        qkpool = ctx.enter_context(tc.tile_pool(name="qk", bufs=2))
        vpool = ctx.enter_context(tc.tile_pool(name="v", bufs=2))
        otpool = ctx.enter_context(tc.tile_pool(name="ot", bufs=2))
        ppool = ctx.enter_context(tc.tile_pool(name="p", bufs=6))
        rpool = ctx.enter_context(tc.tile_pool(name="r", bufs=2))
        bpool = ctx.enter_context(tc.tile_pool(name="bb", bufs=2))
        opool = ctx.enter_context(tc.tile_pool(name="ou", bufs=3))
        ypool = ctx.enter_context(tc.tile_pool(name="y", bufs=8))
        ps_mm = ctx.enter_context(tc.tile_pool(name="psmm", bufs=2,
                                               space="PSUM"))
        ps_st = ctx.enter_context(tc.tile_pool(name="psst", bufs=2,
                                               space="PSUM"))
        ps_ot = ctx.enter_context(tc.tile_pool(name="psot", bufs=2,
                                               space="PSUM"))

        # ---- PE warmup during the initial DMA wait (HAM clock-gate) ----
        zw = const.tile([128, 128], BF16)
        nc.vector.memset(zw[:], 0.0)
        wps = ps_st.tile([128, 1024], F32, tag="st")
        for _ in range(28):
            nc.tensor.matmul(wps[:, 0:128], zw[:], zw[:],
                             start=True, stop=True)

        # ---- weights: wqk + x(chunk0) interleaved on sync (needed first);
        # qb/wv/erp/pb on gpsimd, wp on vector (needed later) ----
        # wqk is stored block-major: tile obi holds output block obi's
        # [128 K x 128 M] slices for all 6 K-blocks side by side, fetched
        # by one 3D-AP DMA each, ordered q0,k0,q1,k1,... so the first QKV
        # groups start as soon as ~2 tiles have landed.
        wv, wp = [], []
        wqkb = [None] * (2 * CB)
        xt_holder = [None]
        xt0 = []
        for cb in range(CB):
            t = xpool.tile([128, chunk_t], BF16, tag=f"xt{cb}")
            nc.scalar.dma_start(t[:], xT_d[cb * 128:(cb + 1) * 128,
                                           0:chunk_t])
            xt0.append(t)
        xt_holder[0] = xt0
        for obi in range(2 * CB):
            ob = obi // 2
            o = (ob if obi % 2 == 0 else ob + CB) * 128
            t = wpool.tile([128, CB * 128], BF16, tag=f"wqkb{obi}")
            src = bass.AP(wqk_d.tensor, wqk_d.offset + o,
                          [[2 * C, 128], [128 * 2 * C, CB], [1, 128]])
            nc.sync.dma_start(t[:], src)
            wqkb[obi] = t
        qb = const.tile([128, CB], F32)
        nc.gpsimd.dma_start(qb[:], qb_d[:, :])
        for cb in range(CB):
            t = wpool.tile([128, C], BF16, tag=f"wv{cb}")
            nc.gpsimd.dma_start(t[:], wv_d[cb * 128:(cb + 1) * 128, :])
            wv.append(t)
        erp = []
        for hp in range(H // 2):
            t = wpool.tile([128, 4 * N], BF16, tag=f"erp{hp}")
            nc.gpsimd.dma_start(t[:], erp_d[hp, :, :])
            erp.append(t)
        pb = const.tile([128, CB], F32)
        nc.gpsimd.dma_start(pb[:], pb_d[:, :])
        for cb in range(CB):
            t = wpool.tile([128, C], BF16, tag=f"wp{cb}")
            nc.gpsimd.dma_start(t[:], wp_d[cb * 128:(cb + 1) * 128, :])
            wp.append(t)

        tslices = [(i * 512, min(512, chunk_t - i * 512))
                   for i in range((chunk_t + 511) // 512)]

        # ---- emit helpers (closures over per-chunk state) ----
        def make_qk(obi, xt, qT, kT):
            def emit():
                qside = (obi % 2 == 0)
                ob = obi // 2
                dst = qT[ob] if qside else kT[ob]
                for (ts, tl) in tslices:
                    pt = ps_mm.tile([128, 512], F32, tag="mm")
                    for cb in range(CB):
                        nc.tensor.matmul(
                            pt[:, 0:tl],
                            wqkb[obi][:, cb * 128:(cb + 1) * 128],
                            xt[cb][:, ts:ts + tl],
                            start=(cb == 0), stop=(cb == CB - 1))
                    if qside:  # q: bias here, scale folded into exp
                        nc.scalar.activation(dst[:, ts:ts + tl], pt[:, 0:tl],
                                             AF.Identity,
                                             bias=qb[:, ob:ob + 1])
                    else:  # k: plain copy/cast on DVE
                        nc.vector.tensor_copy(dst[:, ts:ts + tl], pt[:, 0:tl])
            return emit

        def make_v(w, xt, vtiles):
            def emit():
                wrow = []
                for (moff, mlen) in ((0, 128), (128, 68)):
                    trel = w * N + moff
                    vt = vpool.tile([128, VW], BF16, tag=f"vb{w}_{moff}")
                    for half in range(2):
                        noff = half * 384
                        pv = ps_mm.tile([128, 512], F32, tag="mm")
                        for cb in range(CB):
                            nc.tensor.matmul(
                                pv[0:mlen, 0:384],
                                xt[cb][:, trel:trel + mlen],
                                wv[cb][:, noff:noff + 384],
                                start=(cb == 0), stop=(cb == CB - 1))
                        vt_base = vt[0:mlen, half * 6 * VG:VW]
                        vt_str = bass.AP(vt_base.tensor, vt_base.offset,
                                         [vt_base.ap[0], [VG, 6], [1, HD]])
                        nc.scalar.copy(vt_str, pv[0:mlen, 0:384])
                    ones_base = vt[0:mlen, HD:HD + 1]
                    ones_str = bass.AP(ones_base.tensor, ones_base.offset,
                                       [ones_base.ap[0], [VG, H], [1, 1]])
                    nc.vector.memset(ones_str, 1.0)
                    wrow.append(vt)
                vtiles[w] = wrow
            return emit

        def emit_pair(w, hp, qT, kT, vtiles, otus, castp):
            wq_tok = w * N
            if hp == 0:
                otu_t = opool.tile([65, H * N], BF16, tag="otun")
                otus[w] = otu_t
            otu = otus[w]
            h0 = 2 * hp
            ob = hp
            # pair S tile: h0 in cols 0:392 (bank 0), h1 in 512:904 (bank 1);
            # alternate row halves so LDWEIGHTS targets the row group the
            # previous MATMUL is not using
            st = ps_st.tile([128, 1024], F32, tag="st")
            qh = (qT[ob][0:64, wq_tok:wq_tok + N],
                  qT[ob][64:128, wq_tok:wq_tok + N])
            for blk in range(2):
                c0 = wq_tok + blk * 128
                for hi in range(2):
                    prt = hi * 64
                    nc.tensor.matmul(
                        st[:, hi * 512 + blk * N:hi * 512 + (blk + 1) * N],
                        kT[ob][prt:prt + 64, c0:c0 + 128],
                        qh[hi], start=True, stop=True)
            # one exp for the pair, 2D-AP source skips the 392:512 gap
            p = ppool.tile([128, 4 * N], BF16, tag="p")
            st_src = bass.AP(st.tensor, st.offset,
                             [st[:].ap[0], [512, 2], [1, 2 * N]])
            nc.scalar.activation(p[:], st_src, AF.Exp, scale=0.125)
            nc.vector.tensor_mul(p[:], p[:], erp[hp][:])
            # previous pair's AV cast goes here: after this pair's mul in
            # the DVE FIFO, so AV(this pair) never waits behind it
            if castp[0] is not None:
                castp[0]()
                castp[0] = None
            ot = ps_ot.tile([65, 2 * N], F32, tag="ot")
            for hi in range(2):
                for bi, (moff, mlen) in enumerate(((0, 128), (128, 68))):
                    nc.tensor.matmul(
                        ot[:, hi * N:(hi + 1) * N],
                        vtiles[w][bi][0:mlen, VG * (h0 + hi):
                                      VG * (h0 + hi) + 65],
                        p[0:mlen, hi * 2 * N + bi * N:
                          hi * 2 * N + (bi + 1) * N],
                        start=(bi == 0), stop=(bi == 1))

            def do_cast():
                nc.vector.tensor_copy(otu[:, h0 * N:(h0 + 2) * N], ot[:])
            if hp == CB - 1:  # den(w) gather needs the full otu this slot
                do_cast()
            else:
                castp[0] = do_cast

        def make_den(w, ch, otus, rbbs):
            def emit():
                otu = otus[w]
                srcrow = otu[64:65, 0:H * N]
                den = rpool.tile([H, N], BF16, tag="den")
                nc.gpsimd.dma_start(
                    den[:], bass.AP(srcrow.tensor, srcrow.offset,
                                    [srcrow.ap[0], [N, H], [1, N]]))
                denf = rpool.tile([H, N], F32, tag="denf")
                nc.vector.tensor_copy(denf[:], den[:])
                rec = rpool.tile([H, N], F32, tag="rec")
                nc.vector.reciprocal_approx_fast(rec[:], denf[:])
                recb = rpool.tile([H, N], BF16, tag="recb")
                nc.vector.tensor_copy(recb[:], rec[:])
                scr = rb_scr[(ch * chunk_w + w) % 2, :]
                nc.sync.dma_start(scr, recb[:, :])
                rbb = bpool.tile([64, H * N], BF16, tag="rbb")
                nc.sync.dma_start(
                    rbb[:], bass.AP(scr.tensor, scr.offset,
                                    [[0, 64], [1, H * N]]))
                rbbs[w] = rbb
            return emit

        def make_fin(w, otus, rbbs, ot_sb, h_lo=0, h_hi=H):
            def emit():
                otu, rbb = otus[w], rbbs[w]
                wq_tok = w * N
                for h in range(h_lo, h_hi):
                    ob = h // 2
                    prt = (h % 2) * 64
                    nc.vector.tensor_mul(
                        ot_sb[ob][prt:prt + 64, wq_tok:wq_tok + N],
                        otu[0:64, h * N:(h + 1) * N],
                        rbb[0:64, h * N:(h + 1) * N])
            return emit

        def make_proj_slice(t0, opb, w, nw, ot_sb):
            def emit():
                o = opb * 128
                ts = w * N
                tl = nw * N
                pt = ps_mm.tile([128, 512], F32, tag="mm")
                for ob in range(CB):
                    nc.tensor.matmul(
                        pt[:, 0:tl],
                        wp[ob][:, o:o + 128],
                        ot_sb[ob][:, ts:ts + tl],
                        start=(ob == 0), stop=(ob == CB - 1))
                yt = ypool.tile([128, 2 * N], F32, tag="y")
                nc.scalar.activation(yt[:, 0:tl], pt[:, 0:tl], AF.Identity,
                                     bias=pb[:, opb:opb + 1])
                nc.sync.dma_start(yT_d[o:o + 128, t0 + ts:t0 + ts + tl],
                                  yt[:, 0:tl])
            return emit

        def make_xfetch(ch):
            def emit():
                t0n = ch * chunk_t
                tiles = []
                for cb in range(CB):
                    t = xpool.tile([128, chunk_t], BF16, tag=f"xt{cb}")
                    nc.sync.dma_start(t[:], xT_d[cb * 128:(cb + 1) * 128,
                                                 t0n:t0n + chunk_t])
                    tiles.append(t)
                xt_holder[0] = tiles
            return emit

        def make_prework():
            # next chunk's qT/kT allocation + first two qk blocks + V(w0),
            # emitted during the current chunk's tail so the next chunk's
            # first pairs never wait on fresh q/k copies.
            state = {}

            def p1():
                xtn = xt_holder[0]
                state["xt"] = xtn
                qTn, kTn = [], []
                for obb in range(CB):
                    tq = qkpool.tile([128, chunk_t], BF16, tag=f"qT{obb}")
                    qTn.append(tq)
                for obb in range(CB):
                    tk = qkpool.tile([128, chunk_t + 64], BF16,
                                     tag=f"kT{obb}")
                    nc.vector.memset(tk[:, chunk_t:chunk_t + 64], 0.0)
                    kTn.append(tk)
                state["qT"], state["kT"] = qTn, kTn
                make_qk(0, xtn, qTn, kTn)()

            def p2():
                make_qk(1, state["xt"], state["qT"], state["kT"])()

            def p3():
                vt = {}
                state["vtiles"] = vt
                make_v(0, state["xt"], vt)()
            return state, [p1, p2, p3]

        fin_prev = None  # fin closure for last window of previous chunk
        proj_prev = []  # proj slice closures of previous chunk
        pre_state = None

        for ch in range(nchunk):
            t0 = ch * chunk_t
            last = (ch == nchunk - 1)
            if pre_state is None:  # ch == 0: inline prelude
                xt = xt_holder[0]
                qT, kT = [], []
                for obb in range(CB):
                    t = qkpool.tile([128, chunk_t], BF16, tag=f"qT{obb}")
                    qT.append(t)
                for obb in range(CB):
                    t = qkpool.tile([128, chunk_t + 64], BF16,
                                    tag=f"kT{obb}")
                    nc.vector.memset(t[:, chunk_t:chunk_t + 64], 0.0)
                    kT.append(t)
                vtiles = {}
                prelude = [make_qk(0, xt, qT, kT), make_qk(1, xt, qT, kT),
                           make_v(0, xt, vtiles)]
            else:
                xt, qT, kT = pre_state["xt"], pre_state["qT"], \
                    pre_state["kT"]
                vtiles = pre_state["vtiles"]
                prelude = []
            ot_sb = []
            for obb in range(CB):
                t = otpool.tile([128, chunk_t], BF16, tag=f"ot{obb}")
                ot_sb.append(t)

            otus, rbbs = {}, {}
            castp = [None]
            qk = [make_qk(obi, xt, qT, kT) for obi in range(2 * CB)]
            vws = [make_v(w, xt, vtiles) for w in range(chunk_w)]
            dens = [make_den(w, ch, otus, rbbs) for w in range(chunk_w)]
            fina = [make_fin(w, otus, rbbs, ot_sb, 0, 6)
                    for w in range(chunk_w)]
            finb = [make_fin(w, otus, rbbs, ot_sb, 6, H)
                    for w in range(chunk_w)]
            if last:  # fine slices so the tail can drain per window
                proj_cur = [make_proj_slice(t0, opb, w, 1, ot_sb)
                            for w in range(chunk_w) for opb in range(CB)]
            else:  # 392-wide slices: fewer ACTs/DMAs, same PE cycles
                proj_cur = [make_proj_slice(t0, opb, w, 2, ot_sb)
                            for w in (0, 2) for opb in range(CB)]

            # background work per pair slot (emitted after that pair)
            bg = {}

            def at(w, hp, fn):
                bg.setdefault((w, hp), []).append(fn)

            for i in range(5):  # qk2..qk11 over slots (0,0)..(0,4)
                at(0, i, qk[2 * i + 2])
                at(0, i, qk[2 * i + 3])
            at(0, 5, dens[0])
            if fin_prev is not None:
                at(0, 5, fin_prev[0])
                at(0, 5, fin_prev[1])
            at(0, 5, vws[1])
            if not last:
                at(1, 2, make_xfetch(ch + 1))
            at(2, 0, fina[0])
            at(2, 1, finb[0])
            at(3, 0, fina[1])
            at(3, 1, finb[1])

            # place proj units (prev-chunk slices + last chunk's own w0/w1)
            units = list(proj_prev)
            gates = {}
            if last:
                for u in proj_cur[0:CB]:
                    units.append(u)
                    gates[id(u)] = 8  # after finb[0] at slot (2,1)
                for u in proj_cur[CB:2 * CB]:
                    units.append(u)
                    gates[id(u)] = 14  # after finb[1] at slot (3,1)
            all_slots = [(w, hp) for w in range(1, chunk_w)
                         for hp in range(CB)]
            # non-last chunks reserve (3,2)..(3,4) for next-chunk prework
            skip = {sl for sl in all_slots if sl[1] == 5}
            if not last:
                skip |= {(3, 2), (3, 3), (3, 4)}
            cap = 2 if last else 1
            for si, sl in enumerate(all_slots):
                if sl in skip:
                    continue
                placed = 0
                while units and placed < cap and gates.get(id(units[0]),
                                                           0) <= si:
                    at(sl[0], sl[1], units.pop(0))
                    placed += 1
            assert not units, f"unplaced proj units: {len(units)}"

            at(1, 5, dens[1])
            at(1, 5, vws[2])
            at(2, 5, dens[2])
            at(2, 5, vws[3])
            at(3, 5, dens[3])
            at(3, 5, fina[2])
            at(3, 5, finb[2])
            if not last:
                pre_state, pre_fns = make_prework()
                at(3, 2, pre_fns[0])
                at(3, 3, pre_fns[1])
                at(3, 4, pre_fns[2])

            for fn in prelude:
                fn()

            for w in range(chunk_w):
                for hp in range(CB):
                    emit_pair(w, hp, qT, kT, vtiles, otus, castp)
                    for fn in bg.get((w, hp), ()):
                        fn()

            if last:
                fina[3]()
                finb[3]()
                for fn in proj_cur[2 * CB:]:
                    fn()
            else:
                fin_prev = (fina[3], finb[3])
                proj_prev = proj_cur

    nc.compile()
    return nc


def _host_prep(x, qkv_w, q_bias, v_bias, rpb_table, proj_w, proj_b, rel_index,
               wpc=WPC):
    x = np.asarray(x, np.float32)
    ncores = x.shape[0] // wpc
    t_total = wpc * N
    xT = np.ascontiguousarray(
        x.reshape(ncores, t_total, C).transpose(0, 2, 1)).astype(
            ml_dtypes.bfloat16)
    qkv_w = np.asarray(qkv_w, np.float32)
    wqkT = np.ascontiguousarray(qkv_w[0:2 * C].T).astype(ml_dtypes.bfloat16)
    wvT = np.ascontiguousarray(qkv_w[2 * C:3 * C].T).astype(
        ml_dtypes.bfloat16)
    projwT = np.ascontiguousarray(
        np.asarray(proj_w, np.float32).T).astype(ml_dtypes.bfloat16)
    qbT = np.ascontiguousarray(
        np.asarray(q_bias, np.float32).reshape(CB, 128).T)
    # softmax rows sum to 1, so the v bias contributes proj_w @ v_bias to
    # every output token: fold it into the proj bias
    pb_eff = (np.asarray(proj_b, np.float32)
              + np.asarray(proj_w, np.float32) @ np.asarray(v_bias,
                                                            np.float32))
    pbT = np.ascontiguousarray(pb_eff.reshape(CB, 128).T)
    rel = np.asarray(rel_index).reshape(N, N)
    rpb = np.asarray(rpb_table, np.float32)[rel]              # [n, m, H]
    erp_full = np.exp(rpb).transpose(2, 1, 0)                 # [H, m, n]
    erpT = np.zeros((H // 2, 128, 4 * N), np.float32)
    for hp in range(H // 2):
        for hi in range(2):
            o = hi * 2 * N
            erpT[hp, :, o:o + N] = erp_full[2 * hp + hi, 0:128, :]
            erpT[hp, 0:68, o + N:o + 2 * N] = erp_full[2 * hp + hi,
                                                       128:196, :]
    erpT = erpT.astype(ml_dtypes.bfloat16)
    return xT, wqkT, wvT, projwT, qbT, pbT, erpT


def kernel(x, qkv_w, q_bias, v_bias, rpb_table, proj_w, proj_b, rel_index,
           num_heads=12, _trace=False):
    xT, wqkT, wvT, projwT, qbT, pbT, erpT = _host_prep(
        x, qkv_w, q_bias, v_bias, rpb_table, proj_w, proj_b, rel_index)
    if _trace:
        _install_ntff_hook()
    nc = _NC_CACHE.get("nc")
    if nc is None:
        nc = _build_nc()
        _NC_CACHE["nc"] = nc
    in_maps = [
        {"xT": np.ascontiguousarray(xT[c]), "wqkT": wqkT, "wvT": wvT,
         "projwT": projwT, "qbT": qbT, "pbT": pbT, "erpT": erpT}
        for c in range(NCORES)
    ]
    res = run_bass_kernel_spmd(nc, in_maps, core_ids=list(range(NCORES)),
                               trace=_trace)
    yT = np.stack([res.results[c]["yT"] for c in range(NCORES)])
    out = np.ascontiguousarray(yT.transpose(0, 2, 1)).reshape(B, N, C)
    if _trace:
        kernel._last_exec_time_ns = res.exec_time_ns
        kernel._last_results = res
    return out.astype(np.float32)


# revision 30
# speedup vs baseline: 1.1751x; 1.1751x over previous
# Swin-style window attention (B=256 windows, N=196, C=768, H=12) on 8 trn2
# cores. Data-parallel over windows: 32 windows/core, chunks of 4 windows.
#
# Schedule: everything is woven at (window, head-pair) granularity so no
# engine ever drains. Per chunk: q0/k0 + V(w0) up front, then 24 attention
# pairs with background work attached to each pair slot:
#   w0 pairs carry the remaining 10 qk block GEMMs,
#   w1-w3 pairs carry prev-chunk proj slices (per 128-out-block x window,
#   24 of them), V(w1..w3), x-prefetch for the next chunk, finish-muls and
#   denominator chains.
# Engine assignment: q-copies/exp/proj/V-copies on Scalar, k-copies and
# finish muls and AV casts on DVE, den gather on GpSimd, weight DMAs spread
# over gpsimd/vector queues at startup so the Sync queue starts x/wqk
# immediately. ~28 warmup matmuls run during the initial DMA wait so the
# PE HAM clock-gate is at 2.4 GHz when real work arrives.
#
# Attention per (window, head): S.T = kT_h.T @ qT_h into a per-head
# [128,392] PSUM bank tile (keys 0:128 cols 0:196, keys 128:196 cols
# 196:392), p = exp(0.125*S) * exp(rpb) (one ACT + one DVE mul, 392 cols),
# OT = [V_h | ones].T @ p pair-packed into [65, 392] PSUM (den row 64),
# 1/den via gpsimd strided gather + reciprocal + DRAM-bounce broadcast.
# v_bias is folded into the proj bias on the host (softmax rows sum to 1).
# PSUM banks: mm [128,512]x3, st [128,392]x3, ot [65,392]x2 = 8.
import sys

sys.path.insert(0, "/opt/trn_rl_repo")

from contextlib import ExitStack

import ml_dtypes
import numpy as np

import concourse.bass as bass
import concourse.bacc as bacc
import concourse.mybir as mybir
import concourse.tile as tile
from concourse.bass_utils import run_bass_kernel_spmd

F32 = mybir.dt.float32
BF16 = mybir.dt.bfloat16
AF = mybir.ActivationFunctionType
ALU = mybir.AluOpType

_NC_CACHE = {}
NCORES = 8
B, N, C, H = 256, 196, 768, 12
HD = C // H  # 64
WPC = B // NCORES  # 32 windows per core
T = WPC * N  # 6272 tokens per core
CB = C // 128  # 6 contraction blocks
CHUNK_W = 4  # windows per chunk
VG = HD + 2  # 66: v col group stride (64 v cols + ones col + pad)
VW = H * VG  # 792: strided V tile width


def _install_ntff_hook():
    """Recreate the antenv.axon_hooks shim so trace=True works under axon."""
    import types

    if "antenv.axon_hooks" in sys.modules:
        return
    mod = types.ModuleType("antenv.axon_hooks")
    mod._hook = None
    mod.set_axon_ntff_profile_hook = lambda h: setattr(mod, "_hook", h)
    mod.get_axon_ntff_profile_hook = lambda: mod._hook
    sys.modules["antenv.axon_hooks"] = mod
    try:
        sys.path.insert(0, "/root/.axon_site/trn_agent_boot")
        from trn_boot import _ntff_profile_via_ctypes

        hook = _ntff_profile_via_ctypes("/opt/axon/libaxon_pjrt.so")
        if hook is not None:
            mod._hook = hook
    except Exception:
        pass


def _build_nc(wpc=WPC, chunk_w=CHUNK_W):
    t_total = wpc * N
    nchunk = wpc // chunk_w
    chunk_t = chunk_w * N

    nc = bacc.Bacc("TRN2", target_bir_lowering=False, debug=False,
                   num_devices=NCORES)
    xT_d = nc.dram_tensor("xT", [C, t_total], BF16, kind="ExternalInput").ap()
    wqk_d = nc.dram_tensor("wqkT", [2 * CB, 128, CB * 128], BF16,
                           kind="ExternalInput").ap()
    wv_d = nc.dram_tensor("wvT", [C, C], BF16, kind="ExternalInput").ap()
    wp_d = nc.dram_tensor("projwT", [C, C], BF16, kind="ExternalInput").ap()
    qb_d = nc.dram_tensor("qbT", [128, CB], F32, kind="ExternalInput").ap()
    pb_d = nc.dram_tensor("pbT", [128, CB], F32, kind="ExternalInput").ap()
    erp_d = nc.dram_tensor("erpT", [H // 2, 128, 4 * N], BF16,
                           kind="ExternalInput").ap()
    rb_scr = nc.dram_tensor("rb_scr", [2, H * N], BF16, kind="Internal").ap()
    yT_d = nc.dram_tensor("yT", [C, t_total], F32, kind="ExternalOutput").ap()

    with tile.TileContext(nc) as tc, ExitStack() as ctx:
        const = ctx.enter_context(tc.tile_pool(name="const", bufs=1))
        wpool = ctx.enter_context(tc.tile_pool(name="w", bufs=1))
        xpool = ctx.enter_context(tc.tile_pool(name="x", bufs=2))
        qkpool = ctx.enter_context(tc.tile_pool(name="qk", bufs=2))
        vpool = ctx.enter_context(tc.tile_pool(name="v", bufs=2))
        otpool = ctx.enter_context(tc.tile_pool(name="ot", bufs=2))
        ppool = ctx.enter_context(tc.tile_pool(name="p", bufs=6))
        rpool = ctx.enter_context(tc.tile_pool(name="r", bufs=2))
        bpool = ctx.enter_context(tc.tile_pool(name="bb", bufs=2))
        opool = ctx.enter_context(tc.tile_pool(name="ou", bufs=3))
        ypool = ctx.enter_context(tc.tile_pool(name="y", bufs=8))
        ps_mm = ctx.enter_context(tc.tile_pool(name="psmm", bufs=2,
                                               space="PSUM"))
        ps_st = ctx.enter_context(tc.tile_pool(name="psst", bufs=2,
                                               space="PSUM"))
        ps_ot = ctx.enter_context(tc.tile_pool(name="psot", bufs=2,
                                               space="PSUM"))

        # ---- PE warmup during the initial DMA wait (HAM clock-gate) ----
        zw = const.tile([128, 128], BF16)
        nc.vector.memset(zw[:], 0.0)
        wps = ps_st.tile([128, 1024], F32, tag="st")
        for _ in range(28):
            nc.tensor.matmul(wps[:, 0:128], zw[:], zw[:],
                             start=True, stop=True)

        # ---- weights: wqk + x(chunk0) interleaved on sync (needed first);
        # qb/wv/erp/pb on gpsimd, wp on vector (needed later) ----
        # wqk is stored block-major: tile obi holds output block obi's
        # [128 K x 128 M] slices for all 6 K-blocks side by side, fetched
        # by one 3D-AP DMA each, ordered q0,k0,q1,k1,... so the first QKV
        # groups start as soon as ~2 tiles have landed.
        wv, wp = [], []
        wqkb = [None] * (2 * CB)
        xt_holder = [None]
        xt0 = []
        for cb in range(CB):
            t = xpool.tile([128, chunk_t], BF16, tag=f"xt{cb}")
            nc.scalar.dma_start(t[:], xT_d[cb * 128:(cb + 1) * 128,
                                           0:chunk_t])
            xt0.append(t)
        xt_holder[0] = xt0
        for obi in range(2 * CB):
            t = wpool.tile([128, CB * 128], BF16, tag=f"wqkb{obi}")
            nc.sync.dma_start(t[:], wqk_d[obi, :, :])
            wqkb[obi] = t
        qb = const.tile([128, CB], F32)
        nc.gpsimd.dma_start(qb[:], qb_d[:, :])
        for cb in range(CB):
            t = wpool.tile([128, C], BF16, tag=f"wv{cb}")
            nc.gpsimd.dma_start(t[:], wv_d[cb * 128:(cb + 1) * 128, :])
            wv.append(t)
        erp = []
        for hp in range(H // 2):
            t = wpool.tile([128, 4 * N], BF16, tag=f"erp{hp}")
            nc.gpsimd.dma_start(t[:], erp_d[hp, :, :])
            erp.append(t)
        pb = const.tile([128, CB], F32)
        nc.gpsimd.dma_start(pb[:], pb_d[:, :])
        for cb in range(CB):
            t = wpool.tile([128, C], BF16, tag=f"wp{cb}")
            nc.gpsimd.dma_start(t[:], wp_d[cb * 128:(cb + 1) * 128, :])
            wp.append(t)

        tslices = [(i * 512, min(512, chunk_t - i * 512))
                   for i in range((chunk_t + 511) // 512)]

        # ---- emit helpers (closures over per-chunk state) ----
        def make_qk(obi, xt, qT, kT):
            def emit():
                qside = (obi % 2 == 0)
                ob = obi // 2
                dst = qT[ob] if qside else kT[ob]
                for (ts, tl) in tslices:
                    pt = ps_mm.tile([128, 512], F32, tag="mm")
                    for cb in range(CB):
                        nc.tensor.matmul(
                            pt[:, 0:tl],
                            wqkb[obi][:, cb * 128:(cb + 1) * 128],
                            xt[cb][:, ts:ts + tl],
                            start=(cb == 0), stop=(cb == CB - 1))
                    if qside:  # q: bias here, scale folded into exp
                        nc.scalar.activation(dst[:, ts:ts + tl], pt[:, 0:tl],
                                             AF.Identity,
                                             bias=qb[:, ob:ob + 1])
                    else:  # k: plain copy/cast on DVE
                        nc.vector.tensor_copy(dst[:, ts:ts + tl], pt[:, 0:tl])
            return emit

        def make_v(w, xt, vtiles):
            def emit():
                wrow = []
                for (moff, mlen) in ((0, 128), (128, 68)):
                    trel = w * N + moff
                    vt = vpool.tile([128, VW], BF16, tag=f"vb{w}_{moff}")
                    for half in range(2):
                        noff = half * 384
                        pv = ps_mm.tile([128, 512], F32, tag="mm")
                        for cb in range(CB):
                            nc.tensor.matmul(
                                pv[0:mlen, 0:384],
                                xt[cb][:, trel:trel + mlen],
                                wv[cb][:, noff:noff + 384],
                                start=(cb == 0), stop=(cb == CB - 1))
                        vt_base = vt[0:mlen, half * 6 * VG:VW]
                        vt_str = bass.AP(vt_base.tensor, vt_base.offset,
                                         [vt_base.ap[0], [VG, 6], [1, HD]])
                        nc.scalar.copy(vt_str, pv[0:mlen, 0:384])
                    ones_base = vt[0:mlen, HD:HD + 1]
                    ones_str = bass.AP(ones_base.tensor, ones_base.offset,
                                       [ones_base.ap[0], [VG, H], [1, 1]])
                    nc.vector.memset(ones_str, 1.0)
                    wrow.append(vt)
                vtiles[w] = wrow
            return emit

        def emit_pair(w, hp, qT, kT, vtiles, otus, castp):
            wq_tok = w * N
            if hp == 0:
                otu_t = opool.tile([65, H * N], BF16, tag="otun")
                otus[w] = otu_t
            otu = otus[w]
            h0 = 2 * hp
            ob = hp
            # pair S tile: h0 in cols 0:392 (bank 0), h1 in 512:904 (bank 1);
            # alternate row halves so LDWEIGHTS targets the row group the
            # previous MATMUL is not using
            st = ps_st.tile([128, 1024], F32, tag="st")
            qh = (qT[ob][0:64, wq_tok:wq_tok + N],
                  qT[ob][64:128, wq_tok:wq_tok + N])
            for blk in range(2):
                c0 = wq_tok + blk * 128
                for hi in range(2):
                    prt = hi * 64
                    nc.tensor.matmul(
                        st[:, hi * 512 + blk * N:hi * 512 + (blk + 1) * N],
                        kT[ob][prt:prt + 64, c0:c0 + 128],
                        qh[hi], start=True, stop=True)
            # one exp for the pair, 2D-AP source skips the 392:512 gap
            p = ppool.tile([128, 4 * N], BF16, tag="p")
            st_src = bass.AP(st.tensor, st.offset,
                             [st[:].ap[0], [512, 2], [1, 2 * N]])
            nc.scalar.activation(p[:], st_src, AF.Exp, scale=0.125)
            nc.vector.tensor_mul(p[:], p[:], erp[hp][:])
            # previous pair's AV cast goes here: after this pair's mul in
            # the DVE FIFO, so AV(this pair) never waits behind it
            if castp[0] is not None:
                castp[0]()
                castp[0] = None
            ot = ps_ot.tile([65, 2 * N], F32, tag="ot")
            for hi in range(2):
                for bi, (moff, mlen) in enumerate(((0, 128), (128, 68))):
                    nc.tensor.matmul(
                        ot[:, hi * N:(hi + 1) * N],
                        vtiles[w][bi][0:mlen, VG * (h0 + hi):
                                      VG * (h0 + hi) + 65],
                        p[0:mlen, hi * 2 * N + bi * N:
                          hi * 2 * N + (bi + 1) * N],
                        start=(bi == 0), stop=(bi == 1))

            def do_cast():
                nc.vector.tensor_copy(otu[:, h0 * N:(h0 + 2) * N], ot[:])
            if hp == CB - 1:  # den(w) gather needs the full otu this slot
                do_cast()
            else:
                castp[0] = do_cast

        def make_den(w, ch, otus, rbbs):
            def emit():
                otu = otus[w]
                srcrow = otu[64:65, 0:H * N]
                den = rpool.tile([H, N], BF16, tag="den")
                nc.gpsimd.dma_start(
                    den[:], bass.AP(srcrow.tensor, srcrow.offset,
                                    [srcrow.ap[0], [N, H], [1, N]]))
                denf = rpool.tile([H, N], F32, tag="denf")
                nc.vector.tensor_copy(denf[:], den[:])
                rec = rpool.tile([H, N], F32, tag="rec")
                nc.vector.reciprocal_approx_fast(rec[:], denf[:])
                recb = rpool.tile([H, N], BF16, tag="recb")
                nc.vector.tensor_copy(recb[:], rec[:])
                scr = rb_scr[(ch * chunk_w + w) % 2, :]
                nc.sync.dma_start(scr, recb[:, :])
                rbb = bpool.tile([64, H * N], BF16, tag="rbb")
                nc.sync.dma_start(
                    rbb[:], bass.AP(scr.tensor, scr.offset,
                                    [[0, 64], [1, H * N]]))
                rbbs[w] = rbb
            return emit

        def make_fin(w, otus, rbbs, ot_sb, h_lo=0, h_hi=H):
            def emit():
                otu, rbb = otus[w], rbbs[w]
                wq_tok = w * N
                for h in range(h_lo, h_hi):
                    ob = h // 2
                    prt = (h % 2) * 64
                    nc.vector.tensor_mul(
                        ot_sb[ob][prt:prt + 64, wq_tok:wq_tok + N],
                        otu[0:64, h * N:(h + 1) * N],
                        rbb[0:64, h * N:(h + 1) * N])
            return emit

        def make_proj_slice(t0, opb, w, nw, ot_sb):
            def emit():
                o = opb * 128
                ts = w * N
                tl = nw * N
                pt = ps_mm.tile([128, 512], F32, tag="mm")
                for ob in range(CB):
                    nc.tensor.matmul(
                        pt[:, 0:tl],
                        wp[ob][:, o:o + 128],
                        ot_sb[ob][:, ts:ts + tl],
                        start=(ob == 0), stop=(ob == CB - 1))
                yt = ypool.tile([128, 2 * N], F32, tag="y")
                nc.scalar.activation(yt[:, 0:tl], pt[:, 0:tl], AF.Identity,
                                     bias=pb[:, opb:opb + 1])
                nc.sync.dma_start(yT_d[o:o + 128, t0 + ts:t0 + ts + tl],
                                  yt[:, 0:tl])
            return emit

        def make_xfetch(ch):
            def emit():
                t0n = ch * chunk_t
                tiles = []
                for cb in range(CB):
                    t = xpool.tile([128, chunk_t], BF16, tag=f"xt{cb}")
                    nc.sync.dma_start(t[:], xT_d[cb * 128:(cb + 1) * 128,
                                                 t0n:t0n + chunk_t])
                    tiles.append(t)
                xt_holder[0] = tiles
            return emit

        def make_prework():
            # next chunk's qT/kT allocation + first two qk blocks + V(w0),
            # emitted during the current chunk's tail so the next chunk's
            # first pairs never wait on fresh q/k copies.
            state = {}

            def p1():
                xtn = xt_holder[0]
                state["xt"] = xtn
                qTn, kTn = [], []
                for obb in range(CB):
                    tq = qkpool.tile([128, chunk_t], BF16, tag=f"qT{obb}")
                    qTn.append(tq)
                for obb in range(CB):
                    tk = qkpool.tile([128, chunk_t + 64], BF16,
                                     tag=f"kT{obb}")
                    nc.vector.memset(tk[:, chunk_t:chunk_t + 64], 0.0)
                    kTn.append(tk)
                state["qT"], state["kT"] = qTn, kTn
                make_qk(0, xtn, qTn, kTn)()

            def p2():
                make_qk(1, state["xt"], state["qT"], state["kT"])()

            def p3():
                vt = {}
                state["vtiles"] = vt
                make_v(0, state["xt"], vt)()
            return state, [p1, p2, p3]

        fin_prev = None  # fin closure for last window of previous chunk
        proj_prev = []  # proj slice closures of previous chunk
        pre_state = None

        for ch in range(nchunk):
            t0 = ch * chunk_t
            last = (ch == nchunk - 1)
            if pre_state is None:  # ch == 0: inline prelude
                xt = xt_holder[0]
                qT, kT = [], []
                for obb in range(CB):
                    t = qkpool.tile([128, chunk_t], BF16, tag=f"qT{obb}")
                    qT.append(t)
                for obb in range(CB):
                    t = qkpool.tile([128, chunk_t + 64], BF16,
                                    tag=f"kT{obb}")
                    nc.vector.memset(t[:, chunk_t:chunk_t + 64], 0.0)
                    kT.append(t)
                vtiles = {}
                prelude = [make_qk(0, xt, qT, kT), make_qk(1, xt, qT, kT),
                           make_v(0, xt, vtiles)]
            else:
                xt, qT, kT = pre_state["xt"], pre_state["qT"], \
                    pre_state["kT"]
                vtiles = pre_state["vtiles"]
                prelude = []
            ot_sb = []
            for obb in range(CB):
                t = otpool.tile([128, chunk_t], BF16, tag=f"ot{obb}")
                ot_sb.append(t)

            otus, rbbs = {}, {}
            castp = [None]
            qk = [make_qk(obi, xt, qT, kT) for obi in range(2 * CB)]
            vws = [make_v(w, xt, vtiles) for w in range(chunk_w)]
            dens = [make_den(w, ch, otus, rbbs) for w in range(chunk_w)]
            fina = [make_fin(w, otus, rbbs, ot_sb, 0, 6)
                    for w in range(chunk_w)]
            finb = [make_fin(w, otus, rbbs, ot_sb, 6, H)
                    for w in range(chunk_w)]
            if last:  # fine slices so the tail can drain per window
                proj_cur = [make_proj_slice(t0, opb, w, 1, ot_sb)
                            for w in range(chunk_w) for opb in range(CB)]
            else:  # 392-wide slices: fewer ACTs/DMAs, same PE cycles
                proj_cur = [make_proj_slice(t0, opb, w, 2, ot_sb)
                            for w in (0, 2) for opb in range(CB)]

            # background work per pair slot (emitted after that pair)
            bg = {}

            def at(w, hp, fn):
                bg.setdefault((w, hp), []).append(fn)

            for i in range(5):  # qk2..qk11 over slots (0,0)..(0,4)
                at(0, i, qk[2 * i + 2])
                at(0, i, qk[2 * i + 3])
            at(0, 5, dens[0])
            if fin_prev is not None:
                at(0, 5, fin_prev[0])
                at(0, 5, fin_prev[1])
            at(0, 5, vws[1])
            if not last:
                at(1, 2, make_xfetch(ch + 1))
            at(2, 0, fina[0])
            at(2, 1, finb[0])
            at(3, 0, fina[1])
            at(3, 1, finb[1])

            # place proj units (prev-chunk slices + last chunk's own w0/w1)
            units = list(proj_prev)
            gates = {}
            if last:
                for u in proj_cur[0:CB]:
                    units.append(u)
                    gates[id(u)] = 8  # after finb[0] at slot (2,1)
                for u in proj_cur[CB:2 * CB]:
                    units.append(u)
                    gates[id(u)] = 14  # after finb[1] at slot (3,1)
            all_slots = [(w, hp) for w in range(1, chunk_w)
                         for hp in range(CB)]
            # non-last chunks reserve (3,2)..(3,4) for next-chunk prework
            skip = {sl for sl in all_slots if sl[1] == 5}
            if not last:
                skip |= {(3, 2), (3, 3), (3, 4)}
            cap = 2 if last else 1
            for si, sl in enumerate(all_slots):
                if sl in skip:
                    continue
                placed = 0
                while units and placed < cap and gates.get(id(units[0]),
                                                           0) <= si:
                    at(sl[0], sl[1], units.pop(0))
                    placed += 1
            assert not units, f"unplaced proj units: {len(units)}"

            at(1, 5, dens[1])
            at(1, 5, vws[2])
            at(2, 5, dens[2])
            at(2, 5, vws[3])
            at(3, 5, dens[3])
            at(3, 5, fina[2])
            at(3, 5, finb[2])
            if not last:
                pre_state, pre_fns = make_prework()
                at(3, 2, pre_fns[0])
                at(3, 3, pre_fns[1])
                at(3, 4, pre_fns[2])

            for fn in prelude:
                fn()

            for w in range(chunk_w):
                for hp in range(CB):
                    emit_pair(w, hp, qT, kT, vtiles, otus, castp)
                    for fn in bg.get((w, hp), ()):
                        fn()

            if last:
                fina[3]()
                finb[3]()
                for fn in proj_cur[2 * CB:]:
                    fn()
            else:
                fin_prev = (fina[3], finb[3])
                proj_prev = proj_cur

    nc.compile()
    return nc


def _host_prep(x, qkv_w, q_bias, v_bias, rpb_table, proj_w, proj_b, rel_index,
               wpc=WPC):
    x = np.asarray(x, np.float32)
    ncores = x.shape[0] // wpc
    t_total = wpc * N
    xT = np.ascontiguousarray(
        x.reshape(ncores, t_total, C).transpose(0, 2, 1)).astype(
            ml_dtypes.bfloat16)
    qkv_w = np.asarray(qkv_w, np.float32)
    wqkT = np.ascontiguousarray(qkv_w[0:2 * C].T)  # [C, 2C] (c_in, c_out)
    # block-major for the kernel: tile obi = output block obi's [128 K x
    # 128 M] slices for all 6 K-blocks, ordered q0,k0,q1,k1,...
    wqkb = np.zeros((2 * CB, 128, CB * 128), np.float32)
    for obi in range(2 * CB):
        ob = obi // 2
        o = (ob if obi % 2 == 0 else ob + CB) * 128
        for cb in range(CB):
            wqkb[obi, :, cb * 128:(cb + 1) * 128] = \
                wqkT[cb * 128:(cb + 1) * 128, o:o + 128]
    wqkT = wqkb.astype(ml_dtypes.bfloat16)
    wvT = np.ascontiguousarray(qkv_w[2 * C:3 * C].T).astype(
        ml_dtypes.bfloat16)
    projwT = np.ascontiguousarray(
        np.asarray(proj_w, np.float32).T).astype(ml_dtypes.bfloat16)
    qbT = np.ascontiguousarray(
        np.asarray(q_bias, np.float32).reshape(CB, 128).T)
    # softmax rows sum to 1, so the v bias contributes proj_w @ v_bias to
    # every output token: fold it into the proj bias
    pb_eff = (np.asarray(proj_b, np.float32)
              + np.asarray(proj_w, np.float32) @ np.asarray(v_bias,
                                                            np.float32))
    pbT = np.ascontiguousarray(pb_eff.reshape(CB, 128).T)
    rel = np.asarray(rel_index).reshape(N, N)
    rpb = np.asarray(rpb_table, np.float32)[rel]              # [n, m, H]
    erp_full = np.exp(rpb).transpose(2, 1, 0)                 # [H, m, n]
    erpT = np.zeros((H // 2, 128, 4 * N), np.float32)
    for hp in range(H // 2):
        for hi in range(2):
            o = hi * 2 * N
            erpT[hp, :, o:o + N] = erp_full[2 * hp + hi, 0:128, :]
            erpT[hp, 0:68, o + N:o + 2 * N] = erp_full[2 * hp + hi,
                                                       128:196, :]
    erpT = erpT.astype(ml_dtypes.bfloat16)
    return xT, wqkT, wvT, projwT, qbT, pbT, erpT


def kernel(x, qkv_w, q_bias, v_bias, rpb_table, proj_w, proj_b, rel_index,
           num_heads=12, _trace=False):
    xT, wqkT, wvT, projwT, qbT, pbT, erpT = _host_prep(
        x, qkv_w, q_bias, v_bias, rpb_table, proj_w, proj_b, rel_index)
    if _trace:
        _install_ntff_hook()
    nc = _NC_CACHE.get("nc")
    if nc is None:
        nc = _build_nc()
        _NC_CACHE["nc"] = nc
    in_maps = [
        {"xT": np.ascontiguousarray(xT[c]), "wqkT": wqkT, "wvT": wvT,
         "projwT": projwT, "qbT": qbT, "pbT": pbT, "erpT": erpT}
        for c in range(NCORES)
    ]
    res = run_bass_kernel_spmd(nc, in_maps, core_ids=list(range(NCORES)),
                               trace=_trace)
    yT = np.stack([res.results[c]["yT"] for c in range(NCORES)])
    out = np.ascontiguousarray(yT.transpose(0, 2, 1)).reshape(B, N, C)
    if _trace:
        kernel._last_exec_time_ns = res.exec_time_ns
        kernel._last_results = res
    return out.astype(np.float32)
